# revision 2
# baseline (speedup 1.0000x reference)
#!/usr/bin/env python3
"""Trainium2 Bass kernel for nn_BackboneModule (torsion-angle forward kinematics).

Math (per residue, fully data-parallel over N):
  1. Per-type table lookups (transforms/deps/rigids) by residue_type (21 types)
  2. Build local rigid ops from bb + torsion (cos,sin), compose with transforms
  3. Sequential 7-step kinematic chain with type-dependent parent indices
  4. Gather per-atom frames (24 atoms, type-dependent frame index) and apply

Device mapping:
  - residues on partitions (128/tile-col), RSUB residues per partition per tile
  - per-type table gathers via one-hot matmul on PE (bf16 3-split => fp32-exact)
  - type-dep selects as one-hot mask multiply-accumulate on DVE + GPSIMD
  - PSUM->SBUF copies and layout copies on ScalarE
  - rigid ops held in affine [3,4] layout (rot | trans-col)
"""
import numpy as np
import ml_dtypes
from contextlib import ExitStack

import concourse.bass as bass
from concourse import bacc
import concourse.tile as tile
from concourse import mybir
from concourse.bass_utils import run_bass_kernel_spmd

F32 = mybir.dt.float32
BF16 = mybir.dt.bfloat16
MULT = mybir.AluOpType.mult
ADD = mybir.AluOpType.add
SUB = mybir.AluOpType.subtract
ISEQ = mybir.AluOpType.is_equal

G = 8          # rigid groups
A = 24         # atoms
NT = 21        # residue types
KREP = 3 * NT  # bf16 3-split stacked table rows
NCOL = 220     # packed table columns
N_FULL = 400_000
NCORES = 8
N_PER = N_FULL // NCORES          # 50000
RSUB = 28                          # residues per partition per tile
NPAD = 128 * 392                   # 50176 = per-core padded count
NTILES = 392 // RSUB               # 14

# packed table column offsets
def taff_off(g, i=0, c=0):
    return g * 12 + i * 4 + c
def tm_off(g, j):
    return 96 + g * (g - 1) // 2 + j
def x_off(a, c=0):
    return 124 + a * 3 + c
def rd_off(a):
    return 196 + a


def ap3(t, off, *dims):
    """View into an SBUF tile: partition dim + up to 3 custom free dims [step, count]."""
    b = t[:]
    return bass.AP(b.tensor, b.offset + off, [list(b.ap[0])] + [list(d) for d in dims])


def build_tables(transforms_table, rigids_table, transforms_dep_table, rigids_dep_table):
    """Host-side packing of the tiny per-type tables into one [21, NCOL] fp32 table,
    then bf16 3-split stack [63, NCOL]."""
    T = np.asarray(transforms_table, np.float32)   # [21, 8, 4, 3]
    X = np.asarray(rigids_table, np.float32)       # [21, 24, 3]
    TD = np.asarray(transforms_dep_table)          # [21, 8]
    RD = np.asarray(rigids_dep_table)              # [21, 24]
    tab = np.zeros((NT, NCOL), np.float32)
    # transforms in affine [3,4] layout
    for g in range(G):
        for i in range(3):
            for c in range(3):
                tab[:, taff_off(g, i, c)] = T[:, g, i, c]
            tab[:, taff_off(g, i, 3)] = T[:, g, 3, i]
    # chain parent one-hot masks
    for g in range(1, G):
        for j in range(g):
            tab[:, tm_off(g, j)] = (TD[:, g] == j).astype(np.float32)
    # atom local coords
    for a in range(A):
        for c in range(3):
            tab[:, x_off(a, c)] = X[:, a, c]
    # atom frame indices as floats
    for a in range(A):
        tab[:, rd_off(a)] = RD[:, a].astype(np.float32)
    # bf16 3-split (sum of the three bf16 rows == fp32 value to ~2^-25 rel)
    t0 = tab.astype(ml_dtypes.bfloat16)
    r1 = tab - t0.astype(np.float32)
    t1 = r1.astype(ml_dtypes.bfloat16)
    t2 = (r1 - t1.astype(np.float32)).astype(ml_dtypes.bfloat16)
    return np.concatenate([t0, t1, t2], axis=0)    # [63, NCOL] bf16


def build_program(npad=NPAD, rsub=RSUB, ntiles=NTILES):
    assert npad == 128 * rsub * ntiles
    nc = bacc.Bacc('TRN2', target_bir_lowering=False, debug=False)
    bb_d = nc.dram_tensor("bb", [npad, 12], F32, kind="ExternalInput")
    sc_d = nc.dram_tensor("sc", [npad, 14], F32, kind="ExternalInput")
    p0_d = nc.dram_tensor("pos0", [npad, 3], F32, kind="ExternalInput")
    rt_d = nc.dram_tensor("rt_rep", [KREP, npad], BF16, kind="ExternalInput")
    io_d = nc.dram_tensor("iota63", [KREP, 1], F32, kind="ExternalInput")
    tb_d = nc.dram_tensor("table_b16", [KREP, NCOL], BF16, kind="ExternalInput")
    R_d = nc.dram_tensor("Rout", [npad, 72], F32, kind="ExternalOutput")
    f0_d = nc.dram_tensor("f0out", [npad, 12], F32, kind="ExternalOutput")

    RS = rsub
    NRT = 128 * RS   # residues per tile

    with tile.TileContext(nc) as tc:
        with ExitStack() as ctx:
            cpool = ctx.enter_context(tc.tile_pool(name="const", bufs=1))
            inp = ctx.enter_context(tc.tile_pool(name="inp", bufs=2))
            outp = ctx.enter_context(tc.tile_pool(name="outp", bufs=2))
            work = ctx.enter_context(tc.tile_pool(name="work", bufs=1))
            tmpp = ctx.enter_context(tc.tile_pool(name="tmp", bufs=1))
            psum = ctx.enter_context(tc.tile_pool(name="psum", bufs=8, space="PSUM"))

            io_t = cpool.tile([KREP, 1], F32)
            nc.sync.dma_start(io_t[:], io_d[:])
            tb_t = cpool.tile([KREP, NCOL], BF16)
            nc.sync.dma_start(tb_t[:], tb_d[:])

            for t in range(ntiles):
                n0 = t * NRT
                # ---- loads (p-major: residue = n0 + p*RS + r) ----
                bbT = inp.tile([128, RS * 12], F32, tag="bbT")
                nc.sync.dma_start(
                    bbT[:], bb_d[n0:n0 + NRT, :].rearrange("(p r) f -> p r f", p=128))
                scT = inp.tile([128, RS * 14], F32, tag="scT")
                nc.sync.dma_start(
                    scT[:], sc_d[n0:n0 + NRT, :].rearrange("(p r) f -> p r f", p=128))
                p0T = inp.tile([128, RS * 3], F32, tag="p0T")
                nc.sync.dma_start(
                    p0T[:], p0_d[n0:n0 + NRT, :].rearrange("(p r) f -> p r f", p=128))
                rtT = inp.tile([KREP, NRT], BF16, tag="rtT")
                nc.sync.dma_start(rtT[:], rt_d[:, n0:n0 + NRT])

                # ---- S1: one-hot over types (bf16) ----
                oh = work.tile([KREP, NRT], BF16, tag="oh")
                nc.vector.tensor_scalar(oh[:], rtT[:], io_t[:], None, ISEQ)

                # ---- S2/S3: table gather via PE + PSUM->SBUF copies ----
                GT = work.tile([128, RS * NCOL], F32, tag="GT")
                for r in range(RS):
                    ps = psum.tile([128, NCOL], F32, tag="ps")
                    nc.tensor.matmul(ps[:], oh[:, r::RS], tb_t[:])
                    nc.scalar.copy(GT[:, r * NCOL:(r + 1) * NCOL], ps[:])

                # ---- S4: g0 = compose(T0, bb_affine) -> OPR[0] ----
                OPR = work.tile([128, RS * 96], F32, tag="OPR")
                LOC = work.tile([128, RS * 96], F32, tag="LOC")
                BA = tmpp.tile([128, RS * 12], F32, tag="BA")
                T2 = tmpp.tile([128, RS * 12], F32, tag="T2")
                # bb affine: rot rows copy + trans col = bb[9:12] + pos0
                nc.scalar.copy(ap3(BA, 0, (12, RS), (4, 3), (1, 3)),
                               ap3(bbT, 0, (12, RS), (3, 3), (1, 3)))
                nc.vector.tensor_tensor(ap3(BA, 3, (12, RS), (4, 3)),
                                        ap3(bbT, 9, (12, RS), (1, 3)),
                                        ap3(p0T, 0, (3, RS), (1, 3)), ADD)
                for j in range(3):
                    in0 = ap3(GT, taff_off(0, 0, j), (NCOL, RS), (4, 3), (0, 4))
                    in1 = ap3(BA, j * 4, (12, RS), (0, 3), (1, 4))
                    o = ap3(OPR, 0, (96, RS), (4, 3), (1, 4))
                    if j == 0:
                        nc.vector.tensor_tensor(o, in0, in1, MULT)
                    else:
                        t2v = ap3(T2, 0, (12, RS), (4, 3), (1, 4))
                        nc.vector.tensor_tensor(t2v, in0, in1, MULT)
                        nc.vector.tensor_tensor(o, o, t2v, ADD)
                nc.vector.tensor_tensor(ap3(OPR, 3, (96, RS), (4, 3)),
                                        ap3(OPR, 3, (96, RS), (4, 3)),
                                        ap3(GT, taff_off(0, 0, 3), (NCOL, RS), (4, 3)), ADD)

                # ---- S5: local ops for g=1..7 composed with transforms -> LOC[g] ----
                # LOC[g] col0 = T col0, col3 = T col3 (copies)
                for c in (0, 3):
                    nc.scalar.copy(ap3(LOC, 12 + c, (96, RS), (12, 7), (4, 3)),
                                   ap3(GT, 12 + c, (NCOL, RS), (12, 7), (4, 3)))
                # col1 = Tc1*cos + Tc2*sin ; col2 = Tc2*cos - Tc1*sin
                T5 = tmpp.tile([128, RS * 21], F32, tag="T5")
                T6 = tmpp.tile([128, RS * 21], F32, tag="T6")
                tc1 = ap3(GT, 12 + 1, (NCOL, RS), (12, 7), (4, 3))
                tc2 = ap3(GT, 12 + 2, (NCOL, RS), (12, 7), (4, 3))
                cosv = ap3(scT, 0, (14, RS), (2, 7), (0, 3))
                sinv = ap3(scT, 1, (14, RS), (2, 7), (0, 3))
                t5v = ap3(T5, 0, (21, RS), (3, 7), (1, 3))
                t6v = ap3(T6, 0, (21, RS), (3, 7), (1, 3))
                lc1 = ap3(LOC, 12 + 1, (96, RS), (12, 7), (4, 3))
                lc2 = ap3(LOC, 12 + 2, (96, RS), (12, 7), (4, 3))
                nc.vector.tensor_tensor(t5v, tc1, cosv, MULT)
                nc.gpsimd.tensor_tensor(t6v, tc2, sinv, MULT)
                nc.vector.tensor_tensor(lc1, t5v, t6v, ADD)
                nc.vector.tensor_tensor(t5v, tc2, cosv, MULT)
                nc.gpsimd.tensor_tensor(t6v, tc1, sinv, MULT)
                nc.vector.tensor_tensor(lc2, t5v, t6v, SUB)

                # ---- S6: kinematic chain ----
                Pd = tmpp.tile([128, RS * 12], F32, tag="Pd")
                Pp = tmpp.tile([128, RS * 12], F32, tag="Pp")
                PT = tmpp.tile([128, RS * 12], F32, tag="PT")
                PT2 = tmpp.tile([128, RS * 12], F32, tag="PT2")
                for g in range(1, G):
                    djs = [j for j in range(g) if j % 2 == 0]
                    pjs = [j for j in range(g) if j % 2 == 1]
                    for k, j in enumerate(djs):
                        in0 = ap3(OPR, j * 12, (96, RS), (1, 12))
                        in1 = ap3(GT, tm_off(g, j), (NCOL, RS), (0, 12))
                        o = ap3(Pd, 0, (12, RS), (1, 12))
                        if k == 0:
                            nc.vector.tensor_tensor(o, in0, in1, MULT)
                        else:
                            tv = ap3(PT, 0, (12, RS), (1, 12))
                            nc.vector.tensor_tensor(tv, in0, in1, MULT)
                            nc.vector.tensor_tensor(o, o, tv, ADD)
                    for k, j in enumerate(pjs):
                        in0 = ap3(OPR, j * 12, (96, RS), (1, 12))
                        in1 = ap3(GT, tm_off(g, j), (NCOL, RS), (0, 12))
                        o = ap3(Pp, 0, (12, RS), (1, 12))
                        if k == 0:
                            nc.gpsimd.tensor_tensor(o, in0, in1, MULT)
                        else:
                            tv = ap3(PT2, 0, (12, RS), (1, 12))
                            nc.gpsimd.tensor_tensor(tv, in0, in1, MULT)
                            nc.gpsimd.tensor_tensor(o, o, tv, ADD)
                    if pjs:
                        nc.vector.tensor_tensor(ap3(Pd, 0, (12, RS), (1, 12)),
                                                ap3(Pd, 0, (12, RS), (1, 12)),
                                                ap3(Pp, 0, (12, RS), (1, 12)), ADD)
                    # compose(P, LOC[g]) -> OPR[g]
                    og = ap3(OPR, g * 12, (96, RS), (4, 3), (1, 4))
                    for j in range(3):
                        in0 = ap3(Pd, j, (12, RS), (4, 3), (0, 4))
                        in1 = ap3(LOC, g * 12 + j * 4, (96, RS), (0, 3), (1, 4))
                        if j == 0:
                            nc.vector.tensor_tensor(og, in0, in1, MULT)
                        else:
                            t2v = ap3(T2, 0, (12, RS), (4, 3), (1, 4))
                            nc.vector.tensor_tensor(t2v, in0, in1, MULT)
                            nc.vector.tensor_tensor(og, og, t2v, ADD)
                    nc.vector.tensor_tensor(ap3(OPR, g * 12 + 3, (96, RS), (4, 3)),
                                            ap3(OPR, g * 12 + 3, (96, RS), (4, 3)),
                                            ap3(Pd, 3, (12, RS), (4, 3)), ADD)

                # ---- S7: atom frame masks ----
                RM = work.tile([128, RS * 192], F32, tag="RM")
                for j in range(G):
                    eng = nc.vector if j < 4 else nc.gpsimd
                    eng.tensor_scalar(ap3(RM, j * 24, (192, RS), (1, 24)),
                                      ap3(GT, rd_off(0), (NCOL, RS), (1, 24)),
                                      float(j), None, ISEQ)

                # ---- S8: atoms (4 groups of 6): gather frame + apply ----
                RES = outp.tile([128, RS * 72], F32, tag="RES")
                F0 = outp.tile([128, RS * 12], F32, tag="F0")
                OAd = tmpp.tile([128, RS * 72], F32, tag="OAd")
                OAp = tmpp.tile([128, RS * 72], F32, tag="OAp")
                TD_ = tmpp.tile([128, RS * 72], F32, tag="TD_")
                TP_ = tmpp.tile([128, RS * 72], F32, tag="TP_")
                RT1 = tmpp.tile([128, RS * 18], F32, tag="RT1")
                RT2 = tmpp.tile([128, RS * 18], F32, tag="RT2")
                DJS = [0, 2, 4, 5, 6]
                PJS = [1, 3, 7]
                for gi in range(4):
                    a0 = gi * 6
                    oad = ap3(OAd, 0, (72, RS), (12, 6), (1, 12))
                    oap = ap3(OAp, 0, (72, RS), (12, 6), (1, 12))
                    for k, j in enumerate(DJS):
                        in0 = ap3(OPR, j * 12, (96, RS), (0, 6), (1, 12))
                        in1 = ap3(RM, j * 24 + a0, (192, RS), (1, 6), (0, 12))
                        if k == 0:
                            nc.vector.tensor_tensor(oad, in0, in1, MULT)
                        else:
                            tv = ap3(TD_, 0, (72, RS), (12, 6), (1, 12))
                            nc.vector.tensor_tensor(tv, in0, in1, MULT)
                            nc.vector.tensor_tensor(oad, oad, tv, ADD)
                    for k, j in enumerate(PJS):
                        in0 = ap3(OPR, j * 12, (96, RS), (0, 6), (1, 12))
                        in1 = ap3(RM, j * 24 + a0, (192, RS), (1, 6), (0, 12))
                        if k == 0:
                            nc.gpsimd.tensor_tensor(oap, in0, in1, MULT)
                        else:
                            tv = ap3(TP_, 0, (72, RS), (12, 6), (1, 12))
                            nc.gpsimd.tensor_tensor(tv, in0, in1, MULT)
                            nc.gpsimd.tensor_tensor(oap, oap, tv, ADD)
                    nc.vector.tensor_tensor(oad, oad, oap, ADD)
                    if gi == 0:
                        # frame for atom 0, back to original [4,3] layout
                        nc.scalar.copy(ap3(F0, 0, (12, RS), (3, 3), (1, 3)),
                                       ap3(OAd, 0, (72, RS), (4, 3), (1, 3)))
                        nc.scalar.copy(ap3(F0, 9, (12, RS), (1, 3)),
                                       ap3(OAd, 3, (72, RS), (4, 3)))
                    # rotate + translate: R = OA_rot @ x + OA_trans
                    r1v = ap3(RT1, 0, (18, RS), (3, 6), (1, 3))
                    r2v = ap3(RT2, 0, (18, RS), (3, 6), (1, 3))
                    for c in range(3):
                        in0 = ap3(OAd, c, (72, RS), (12, 6), (4, 3))
                        in1 = ap3(GT, x_off(a0, c), (NCOL, RS), (3, 6), (0, 3))
                        if c == 0:
                            nc.vector.tensor_tensor(r1v, in0, in1, MULT)
                        else:
                            nc.vector.tensor_tensor(r2v, in0, in1, MULT)
                            nc.vector.tensor_tensor(r1v, r1v, r2v, ADD)
                    nc.vector.tensor_tensor(
                        ap3(RES, a0 * 3, (72, RS), (3, 6), (1, 3)),
                        r1v, ap3(OAd, 3, (72, RS), (12, 6), (4, 3)), ADD)

                # ---- S9: stores ----
                nc.sync.dma_start(
                    R_d[n0:n0 + NRT, :].rearrange("(p r) f -> p r f", p=128), RES[:])
                nc.sync.dma_start(
                    f0_d[n0:n0 + NRT, :].rearrange("(p r) f -> p r f", p=128), F0[:])
    nc.compile()
    return nc


_PROGRAM_CACHE = {}


def _get_program(npad, rsub, ntiles):
    key = (npad, rsub, ntiles)
    if key not in _PROGRAM_CACHE:
        _PROGRAM_CACHE[key] = build_program(npad, rsub, ntiles)
    return _PROGRAM_CACHE[key]


def kernel(bb, sc, pos0, transforms_table, rigids_table,
           residue_type, transforms_dep_table, rigids_dep_table,
           _trace=False):
    bb = np.asarray(bb, np.float32).reshape(N_FULL, 12)
    sc = np.asarray(sc, np.float32).reshape(N_FULL, 14)
    pos0 = np.asarray(pos0, np.float32).reshape(N_FULL, 3)
    rt = np.asarray(residue_type, np.int32)

    table_b16 = build_tables(transforms_table, rigids_table,
                             transforms_dep_table, rigids_dep_table)
    iota63 = (np.arange(KREP) % NT).astype(np.float32)[:, None]

    nc = _get_program(NPAD, RSUB, NTILES)

    in_maps = []
    for c in range(NCORES):
        s = c * N_PER
        e = s + N_PER
        pad = NPAD - N_PER
        bb_c = np.concatenate([bb[s:e], np.zeros((pad, 12), np.float32)], axis=0)
        sc_c = np.concatenate([sc[s:e], np.zeros((pad, 14), np.float32)], axis=0)
        p0_c = np.concatenate([pos0[s:e], np.zeros((pad, 3), np.float32)], axis=0)
        rt_c = np.concatenate([rt[s:e], np.zeros(pad, np.int32)], axis=0)
        rt_rep = np.broadcast_to(
            rt_c.astype(ml_dtypes.bfloat16)[None, :], (KREP, NPAD)).copy()
        in_maps.append(dict(bb=bb_c, sc=sc_c, pos0=p0_c, rt_rep=rt_rep,
                            iota63=iota63, table_b16=table_b16))

    res = run_bass_kernel_spmd(nc, in_maps, list(range(NCORES)), trace=_trace)

    R = np.empty((N_FULL, A, 3), np.float32)
    f0 = np.empty((N_FULL, 4, 3), np.float32)
    for c in range(NCORES):
        s = c * N_PER
        R[s:s + N_PER] = res.results[c]["Rout"][:N_PER].reshape(N_PER, A, 3)
        f0[s:s + N_PER] = res.results[c]["f0out"][:N_PER].reshape(N_PER, 4, 3)
    kernel.last_exec_time_ns = getattr(res, "exec_time_ns", None)
    kernel.last_results = res
    return R, f0


# revision 16
# speedup vs baseline: 1.2179x; 1.2179x over previous
#!/usr/bin/env python3
"""Trainium2 Bass kernel for nn_BackboneModule (torsion-angle forward kinematics).

Math (per residue, fully data-parallel over N):
  1. Per-type table lookups (transforms/deps/rigids) by residue_type (21 types)
  2. Build local rigid ops from bb + torsion (cos,sin), compose with transforms
  3. Sequential 7-step kinematic chain with type-dependent parent indices
  4. Gather per-atom frames (24 atoms, type-dependent frame index) and apply

Device mapping:
  - residues on partitions (128/tile-col), RSUB residues per partition per tile
  - per-type table gathers via one-hot matmul on PE (bf16 3-split => fp32-exact)
  - type-dep selects as one-hot mask multiply-accumulate on DVE + GPSIMD
  - PSUM->SBUF copies and layout copies on ScalarE
  - rigid ops held in affine [3,4] layout (rot | trans-col)
"""
import numpy as np
import ml_dtypes
from contextlib import ExitStack

import concourse.bass as bass
from concourse import bacc
import concourse.tile as tile
from concourse import mybir
from concourse.bass_utils import run_bass_kernel_spmd

F32 = mybir.dt.float32
BF16 = mybir.dt.bfloat16
MULT = mybir.AluOpType.mult
ADD = mybir.AluOpType.add
SUB = mybir.AluOpType.subtract
ISEQ = mybir.AluOpType.is_equal

G = 8          # rigid groups
A = 24         # atoms
NT = 21        # residue types
KREP = 3 * NT  # bf16 3-split stacked table rows
NCOL = 220     # packed table columns
N_FULL = 400_000
NCORES = 8
N_PER = N_FULL // NCORES          # 50000
RSUB = 14                          # residues per partition per tile
NPAD = 128 * 392                   # 50176 = per-core padded count
NTILES = 392 // RSUB               # 14

# packed table column offsets
def taff_off(g, i=0, c=0):
    return g * 12 + i * 4 + c
def tm_off(g, j):
    return 96 + g * (g - 1) // 2 + j
def x_off(a, c=0):
    return 124 + a * 3 + c
def rd_off(a):
    return 196 + a


def ap3(t, off, *dims):
    """View into an SBUF tile: partition dim + up to 3 custom free dims [step, count]."""
    b = t[:]
    return bass.AP(b.tensor, b.offset + off, [list(b.ap[0])] + [list(d) for d in dims])


def build_tables(transforms_table, rigids_table, transforms_dep_table, rigids_dep_table):
    """Host-side packing of the tiny per-type tables into one [21, NCOL] fp32 table,
    then bf16 3-split stack [63, NCOL]."""
    T = np.asarray(transforms_table, np.float32)   # [21, 8, 4, 3]
    X = np.asarray(rigids_table, np.float32)       # [21, 24, 3]
    TD = np.asarray(transforms_dep_table)          # [21, 8]
    RD = np.asarray(rigids_dep_table)              # [21, 24]
    tab = np.zeros((NT, NCOL), np.float32)
    # transforms in affine [3,4] layout
    for g in range(G):
        for i in range(3):
            for c in range(3):
                tab[:, taff_off(g, i, c)] = T[:, g, i, c]
            tab[:, taff_off(g, i, 3)] = T[:, g, 3, i]
    # chain parent one-hot masks
    for g in range(1, G):
        for j in range(g):
            tab[:, tm_off(g, j)] = (TD[:, g] == j).astype(np.float32)
    # atom local coords
    for a in range(A):
        for c in range(3):
            tab[:, x_off(a, c)] = X[:, a, c]
    # atom frame indices as floats
    for a in range(A):
        tab[:, rd_off(a)] = RD[:, a].astype(np.float32)
    # bf16 3-split (sum of the three bf16 rows == fp32 value to ~2^-25 rel)
    t0 = tab.astype(ml_dtypes.bfloat16)
    r1 = tab - t0.astype(np.float32)
    t1 = r1.astype(ml_dtypes.bfloat16)
    t2 = (r1 - t1.astype(np.float32)).astype(ml_dtypes.bfloat16)
    return np.concatenate([t0, t1, t2], axis=0)    # [63, NCOL] bf16


def build_program(npad=NPAD, rsub=RSUB, ntiles=NTILES, ablate=()):
    assert npad == 128 * rsub * ntiles
    nc = bacc.Bacc('TRN2', target_bir_lowering=False, debug=False)
    bb_d = nc.dram_tensor("bb", [npad, 12], F32, kind="ExternalInput")
    sc_d = nc.dram_tensor("sc", [npad, 14], F32, kind="ExternalInput")
    p0_d = nc.dram_tensor("pos0", [npad, 3], F32, kind="ExternalInput")
    rt_d = nc.dram_tensor("rt_rep", [KREP, npad], BF16, kind="ExternalInput")
    io_d = nc.dram_tensor("iota63", [KREP, 1], F32, kind="ExternalInput")
    tb_d = nc.dram_tensor("table_b16", [KREP, NCOL], BF16, kind="ExternalInput")
    R_d = nc.dram_tensor("Rout", [npad, 72], F32, kind="ExternalOutput")
    f0_d = nc.dram_tensor("f0out", [npad, 12], F32, kind="ExternalOutput")

    RS = rsub
    NRT = 128 * RS   # residues per tile

    with tile.TileContext(nc) as tc:
        with ExitStack() as ctx:
            cpool = ctx.enter_context(tc.tile_pool(name="const", bufs=1))
            inp = ctx.enter_context(tc.tile_pool(name="inp", bufs=2))
            outp = ctx.enter_context(tc.tile_pool(name="outp", bufs=2))
            work = ctx.enter_context(tc.tile_pool(name="work", bufs=2))
            tmpp = ctx.enter_context(tc.tile_pool(name="tmp", bufs=2))
            work2 = ctx.enter_context(tc.tile_pool(name="work2", bufs=2))
            psum = ctx.enter_context(tc.tile_pool(name="psum", bufs=8, space="PSUM"))

            io_t = cpool.tile([KREP, 1], F32)
            nc.sync.dma_start(io_t[:], io_d[:])
            tb_t = cpool.tile([KREP, NCOL], BF16)
            nc.sync.dma_start(tb_t[:], tb_d[:])

            for t in range(ntiles):
                n0 = t * NRT
                # ---- loads (p-major: residue = n0 + p*RS + r) ----
                bbT = inp.tile([128, RS * 12], F32, tag="bbT")
                nc.sync.dma_start(
                    bbT[:], bb_d[n0:n0 + NRT, :].rearrange("(p r) f -> p r f", p=128))
                scT = inp.tile([128, RS * 14], F32, tag="scT")
                nc.sync.dma_start(
                    scT[:], sc_d[n0:n0 + NRT, :].rearrange("(p r) f -> p r f", p=128))
                p0T = inp.tile([128, RS * 3], F32, tag="p0T")
                nc.sync.dma_start(
                    p0T[:], p0_d[n0:n0 + NRT, :].rearrange("(p r) f -> p r f", p=128))
                rtT = inp.tile([KREP, NRT], BF16, tag="rtT")
                nc.sync.dma_start(rtT[:], rt_d[:, n0:n0 + NRT])

                # ---- S1: one-hot over types (bf16) ----
                oh = work.tile([KREP, NRT], BF16, tag="oh")
                nc.vector.tensor_scalar(oh[:], rtT[:], io_t[:], None, ISEQ)

                # ---- S2/S3: table gather via PE + PSUM->SBUF copies ----
                GT = work.tile([128, RS * NCOL], F32, tag="GT")
                for r in range(RS):
                    if 'gather' in ablate: break
                    ps = psum.tile([128, NCOL], F32, tag="ps")
                    nc.tensor.matmul(ps[:], oh[:, r::RS], tb_t[:])
                    nc.scalar.copy(GT[:, r * NCOL:(r + 1) * NCOL], ps[:])

                # ---- S4: g0 = compose(T0, bb_affine) -> OPR[0] ----
                OPR = work2.tile([128, RS * 96], F32, tag="OPR")
                LOC = work.tile([128, RS * 96], F32, tag="LOC")
                BA = tmpp.tile([128, RS * 12], F32, tag="BA")
                T2 = tmpp.tile([128, RS * 12], F32, tag="T2")
                # bb affine: rot rows copy + trans col = bb[9:12] + pos0
                nc.scalar.copy(ap3(BA, 0, (12, RS), (4, 3), (1, 3)),
                               ap3(bbT, 0, (12, RS), (3, 3), (1, 3)))
                nc.vector.tensor_tensor(ap3(BA, 3, (12, RS), (4, 3)),
                                        ap3(bbT, 9, (12, RS), (1, 3)),
                                        ap3(p0T, 0, (3, RS), (1, 3)), ADD)
                for j in range(3):
                    in0 = ap3(GT, taff_off(0, 0, j), (NCOL, RS), (4, 3), (0, 4))
                    in1 = ap3(BA, j * 4, (12, RS), (0, 3), (1, 4))
                    o = ap3(OPR, 0, (96, RS), (4, 3), (1, 4))
                    if j == 0:
                        nc.vector.tensor_tensor(o, in0, in1, MULT)
                    else:
                        t2v = ap3(T2, 0, (12, RS), (4, 3), (1, 4))
                        nc.vector.tensor_tensor(t2v, in0, in1, MULT)
                        nc.vector.tensor_tensor(o, o, t2v, ADD)
                nc.vector.tensor_tensor(ap3(OPR, 3, (96, RS), (4, 3)),
                                        ap3(OPR, 3, (96, RS), (4, 3)),
                                        ap3(GT, taff_off(0, 0, 3), (NCOL, RS), (4, 3)), ADD)

                # ---- S5: local ops for g=1..7 composed with transforms -> LOC[g] ----
                # LOC[g] col0 = T col0, col3 = T col3 (copies)
                for c in (0, 3):
                    nc.scalar.copy(ap3(LOC, 12 + c, (96, RS), (12, 7), (4, 3)),
                                   ap3(GT, 12 + c, (NCOL, RS), (12, 7), (4, 3)))
                # col1 = Tc1*cos + Tc2*sin ; col2 = Tc2*cos - Tc1*sin
                T5 = tmpp.tile([128, RS * 21], F32, tag="T5")
                T6 = tmpp.tile([128, RS * 21], F32, tag="T6")
                tc1 = ap3(GT, 12 + 1, (NCOL, RS), (12, 7), (4, 3))
                tc2 = ap3(GT, 12 + 2, (NCOL, RS), (12, 7), (4, 3))
                cosv = ap3(scT, 0, (14, RS), (2, 7), (0, 3))
                sinv = ap3(scT, 1, (14, RS), (2, 7), (0, 3))
                t5v = ap3(T5, 0, (21, RS), (3, 7), (1, 3))
                t6v = ap3(T6, 0, (21, RS), (3, 7), (1, 3))
                lc1 = ap3(LOC, 12 + 1, (96, RS), (12, 7), (4, 3))
                lc2 = ap3(LOC, 12 + 2, (96, RS), (12, 7), (4, 3))
                nc.gpsimd.tensor_tensor(t5v, tc1, cosv, MULT)
                nc.gpsimd.tensor_tensor(t6v, tc2, sinv, MULT)
                nc.gpsimd.tensor_tensor(lc1, t5v, t6v, ADD)
                nc.gpsimd.tensor_tensor(t5v, tc2, cosv, MULT)
                nc.gpsimd.tensor_tensor(t6v, tc1, sinv, MULT)
                nc.gpsimd.tensor_tensor(lc2, t5v, t6v, SUB)

                # ---- S6: kinematic chain ----
                Pd = tmpp.tile([128, RS * 16], F32, tag="Pd")
                TM8 = work.tile([128, RS * 28], mybir.dt.int8, tag="TM8")
                nc.gpsimd.tensor_copy(
                    ap3(TM8, 0, (28, RS), (1, 28)),
                    ap3(GT, 96, (NCOL, RS), (1, 28)))
                for g in range(1, G):
                    if 'chain' in ablate: break
                    if g == 1:
                        pview = lambda off, *d: ap3(OPR, off, (96, RS), *d)
                    else:
                        pview = lambda off, *d: ap3(Pd, off, (16, RS), *d)
                        # gather parent frame: start from frame 0, predicated-
                        # overwrite with frame j where tmask[g,j] is set
                        nc.scalar.copy(ap3(Pd, 0, (16, RS), (1, 12)),
                                       ap3(OPR, 0, (96, RS), (1, 12)))
                        for j in range(1, g):
                            nc.vector.copy_predicated(
                                ap3(Pd, 0, (16, RS), (1, 12)),
                                ap3(TM8, tm_off(g, j) - 96, (28, RS), (0, 12)),
                                ap3(OPR, j * 12, (96, RS), (1, 12)))
                    # compose(P, LOC[g]) -> OPR[g]  (on POOL)
                    og = ap3(OPR, g * 12, (96, RS), (4, 3), (1, 4))
                    for j in range(3):
                        in0 = pview(j, (4, 3), (0, 4))
                        in1 = ap3(LOC, g * 12 + j * 4, (96, RS), (0, 3), (1, 4))
                        if j == 0:
                            nc.gpsimd.tensor_tensor(og, in0, in1, MULT)
                        else:
                            t2v = ap3(T2, 0, (12, RS), (4, 3), (1, 4))
                            nc.gpsimd.tensor_tensor(t2v, in0, in1, MULT)
                            nc.gpsimd.tensor_tensor(og, og, t2v, ADD)
                    nc.gpsimd.tensor_tensor(ap3(OPR, g * 12 + 3, (96, RS), (4, 3)),
                                            ap3(OPR, g * 12 + 3, (96, RS), (4, 3)),
                                            pview(3, (4, 3)), ADD)

                # ---- S7: atom frame masks ----
                RM = work.tile([128, RS * 192], mybir.dt.int8, tag="RM")
                for j in range(G):
                    eng = nc.gpsimd
                    eng.tensor_scalar(ap3(RM, j * 24, (192, RS), (1, 24)),
                                      ap3(GT, rd_off(0), (NCOL, RS), (1, 24)),
                                      float(j), None, ISEQ)

                # ---- S8: atoms: gather frame via predicated copies + apply ----
                RES = outp.tile([128, RS * 72], F32, tag="RES")
                F0 = outp.tile([128, RS * 12], F32, tag="F0")
                OAd = tmpp.tile([128, RS * 316], F32, tag="OAd")
                RT1 = tmpp.tile([128, RS * 72], F32, tag="RT1")
                RT2 = tmpp.tile([128, RS * 72], F32, tag="RT2")
                if 'atoms' not in ablate:
                    oa_out = ap3(OAd, 0, (316, RS), (13, A), (1, 12))
                    nc.scalar.copy(
                        oa_out, ap3(OPR, 0, (96, RS), (0, A), (1, 12)))
                    for j in range(1, G):
                        nc.vector.copy_predicated(
                            oa_out,
                            ap3(RM, j * 24, (192, RS), (1, A), (0, 12)),
                            ap3(OPR, j * 12, (96, RS), (0, A), (1, 12)))
                    # frame for atom 0, back to original [4,3] layout
                    nc.scalar.copy(ap3(F0, 0, (12, RS), (3, 3), (1, 3)),
                                   ap3(OAd, 0, (316, RS), (4, 3), (1, 3)))
                    nc.scalar.copy(ap3(F0, 9, (12, RS), (1, 3)),
                                   ap3(OAd, 3, (316, RS), (4, 3)))
                    # rotate + translate: R = OA_rot @ x + OA_trans  (on POOL)
                    r1v = ap3(RT1, 0, (72, RS), (3, A), (1, 3))
                    r2v = ap3(RT2, 0, (72, RS), (3, A), (1, 3))
                    for c in range(3):
                        in0 = ap3(OAd, c, (316, RS), (13, A), (4, 3))
                        in1 = ap3(GT, x_off(0, c), (NCOL, RS), (3, A), (0, 3))
                        if c == 0:
                            nc.vector.tensor_tensor(r1v, in0, in1, MULT)
                        else:
                            nc.vector.tensor_tensor(r2v, in0, in1, MULT)
                            nc.vector.tensor_tensor(r1v, r1v, r2v, ADD)
                    nc.vector.tensor_tensor(
                        ap3(RES, 0, (72, RS), (3, A), (1, 3)),
                        r1v, ap3(OAd, 3, (316, RS), (13, A), (4, 3)), ADD)
                else:
                    nc.vector.tensor_copy(RES[:], ap3(OPR, 0, (96, RS), (0, 6), (1, 12)))
                    nc.vector.tensor_copy(F0[:], ap3(OPR, 0, (96, RS), (1, 12)))

                # ---- S9: stores ----
                nc.sync.dma_start(
                    R_d[n0:n0 + NRT, :].rearrange("(p r) f -> p r f", p=128), RES[:])
                nc.sync.dma_start(
                    f0_d[n0:n0 + NRT, :].rearrange("(p r) f -> p r f", p=128), F0[:])
    nc.compile()
    return nc


_PROGRAM_CACHE = {}


def _get_program(npad, rsub, ntiles):
    key = (npad, rsub, ntiles)
    if key not in _PROGRAM_CACHE:
        _PROGRAM_CACHE[key] = build_program(npad, rsub, ntiles)
    return _PROGRAM_CACHE[key]


def kernel(bb, sc, pos0, transforms_table, rigids_table,
           residue_type, transforms_dep_table, rigids_dep_table,
           _trace=False):
    bb = np.asarray(bb, np.float32).reshape(N_FULL, 12)
    sc = np.asarray(sc, np.float32).reshape(N_FULL, 14)
    pos0 = np.asarray(pos0, np.float32).reshape(N_FULL, 3)
    rt = np.asarray(residue_type, np.int32)

    table_b16 = build_tables(transforms_table, rigids_table,
                             transforms_dep_table, rigids_dep_table)
    iota63 = (np.arange(KREP) % NT).astype(np.float32)[:, None]

    nc = _get_program(NPAD, RSUB, NTILES)

    in_maps = []
    for c in range(NCORES):
        s = c * N_PER
        e = s + N_PER
        pad = NPAD - N_PER
        bb_c = np.concatenate([bb[s:e], np.zeros((pad, 12), np.float32)], axis=0)
        sc_c = np.concatenate([sc[s:e], np.zeros((pad, 14), np.float32)], axis=0)
        p0_c = np.concatenate([pos0[s:e], np.zeros((pad, 3), np.float32)], axis=0)
        rt_c = np.concatenate([rt[s:e], np.zeros(pad, np.int32)], axis=0)
        rt_rep = np.broadcast_to(
            rt_c.astype(ml_dtypes.bfloat16)[None, :], (KREP, NPAD)).copy()
        in_maps.append(dict(bb=bb_c, sc=sc_c, pos0=p0_c, rt_rep=rt_rep,
                            iota63=iota63, table_b16=table_b16))

    res = run_bass_kernel_spmd(nc, in_maps, list(range(NCORES)), trace=_trace)

    R = np.empty((N_FULL, A, 3), np.float32)
    f0 = np.empty((N_FULL, 4, 3), np.float32)
    for c in range(NCORES):
        s = c * N_PER
        R[s:s + N_PER] = res.results[c]["Rout"][:N_PER].reshape(N_PER, A, 3)
        f0[s:s + N_PER] = res.results[c]["f0out"][:N_PER].reshape(N_PER, 4, 3)
    kernel.last_exec_time_ns = getattr(res, "exec_time_ns", None)
    kernel.last_results = res
    return R, f0
